# revision 9
# baseline (speedup 1.0000x reference)
"""AttnDecoderRNN (single GRU step + general attention + output head) on 8 trn2 cores.

Data-parallel over batch B=128 -> 16 per core; weights replicated.

Precision strategy: fp32 PE matmuls cost 4 cycles/row on trn2; bf16 costs 1.
Everything upstream of the softmax needs ~1e-4 absolute accuracy on the
energies (softmax exponentiates absolute errors), so those matmuls use an
exact bf16 hi+lo split (x = hi + lo, both bf16) and compute
  x@W ~= hi@W_hi + hi@W_lo + lo@W_hi   (3 bf16 passes, fp32 PSUM accumulate)
dropping only the lo@lo term (~1e-5 rel). The post-softmax path (context,
concat/output head) tolerates relative error, so it runs plain bf16 1-pass.

Math (per core, b = local batch 0..15):
  gi = x @ W_ih.T ; gh = h @ W_hh.T          (PSUM-accumulated matmuls)
  r = sig(gi_r+gh_r+b_r), z = sig(gi_z+gh_z+b_z), n = tanh(i_n+b_in + r*(h_n+b_hn))
  rnn = n + z*(h-n)
  q = rnn @ W_attn                            (b_attn shifts energies per-b only,
                                               softmax-invariant -> dropped)
  e[b,s] = sum_d enc[s,b,d] * q[b,d]          (== reference energies up to const)
  w = softmax(e) ; ctx[b,:] = sum_s w[b,s] enc[s,b,:]
  out = sig(tanh([rnn,ctx] @ W_cat.T + b_cat) @ W_out.T + b_out)
"""

import numpy as np
from contextlib import ExitStack

import concourse.bass as bass
import concourse.tile as tile
from concourse import bacc, mybir
from concourse.bass import ts, ds
from concourse.bass_utils import run_bass_kernel_spmd
from concourse.masks import make_identity

B, S, H = 128, 512, 1024
NCORES = 8
BS = B // NCORES  # 16
F32 = mybir.dt.float32
BF16 = mybir.dt.bfloat16
AF = mybir.ActivationFunctionType
OP = mybir.AluoOpType if hasattr(mybir, "AluoOpType") else mybir.AluOpType

_cached = {}


def _build_kernel(tc: tile.TileContext, io: dict):
    nc = tc.nc
    with ExitStack() as ctx:
        const = ctx.enter_context(tc.tile_pool(name="const", bufs=1))
        sb = ctx.enter_context(tc.tile_pool(name="sb", bufs=1))
        wpool = ctx.enter_context(tc.tile_pool(name="w", bufs=6))
        epool = ctx.enter_context(tc.tile_pool(name="enc", bufs=4))
        rows = ctx.enter_context(tc.tile_pool(name="rows", bufs=3))

        ident = const.tile([128, 128], BF16)
        make_identity(nc, ident[:])

        # ---- small inputs (xT/hT pre-split hi/lo on host)
        def load_xh(name):
            t = sb.tile([128, 8, BS], BF16, tag=name)
            nc.sync.dma_start(t[:], io[name].rearrange("(c p) b -> p c b", p=128))
            return t

        xT_h, xT_l = load_xh("xT_hi"), load_xh("xT_lo")
        hT_h, hT_l = load_xh("hT_hi"), load_xh("hT_lo")
        h_nat = sb.tile([BS, H], F32)
        nc.sync.dma_start(h_nat[:], io["h_nat"][:])
        biases = {}
        for bn, width in [("bias_rz", 2 * H), ("bias_in", H), ("bias_hn", H),
                          ("bias_cat", H), ("bias_out", H)]:
            biases[bn] = sb.tile([BS, width], F32, tag=bn, name=bn)
            nc.sync.dma_start(biases[bn][:], io[bn][:])

        # padded (128-partition) staging tiles for PE transposes
        rnn_h = sb.tile([128, H], BF16, tag="rnn_h")
        rnn_l = sb.tile([128, H], BF16, tag="rnn_l")
        q_h = sb.tile([128, H], BF16, tag="q_h")
        q_l = sb.tile([128, H], BF16, tag="q_l")
        w_16 = sb.tile([128, S], BF16, tag="w_16")
        ctx_16 = sb.tile([128, H], BF16, tag="ctx_16")
        cat_16 = sb.tile([128, H], BF16, tag="cat_16")
        for t in (rnn_h, rnn_l, q_h, q_l, w_16, ctx_16, cat_16):
            nc.vector.memset(t[:], 0.0)

        # ================= Phase A: GRU =================
        with tc.tile_pool(name="psA", bufs=1, space="PSUM") as psA:
            ps_r = psA.tile([BS, H], F32, tag="r")
            ps_z = psA.tile([BS, H], F32, tag="z")
            ps_in = psA.tile([BS, H], F32, tag="in")
            ps_hn = psA.tile([BS, H], F32, tag="hn")
            for c in range(8):
                wih_h = wpool.tile([128, 3 * H], BF16, tag="w")
                nc.sync.dma_start(wih_h[:], io["wihT_hi"][ts(c, 128), :])
                wih_l = wpool.tile([128, 3 * H], BF16, tag="w")
                nc.sync.dma_start(wih_l[:], io["wihT_lo"][ts(c, 128), :])
                whh_h = wpool.tile([128, 3 * H], BF16, tag="w")
                nc.sync.dma_start(whh_h[:], io["whhT_hi"][ts(c, 128), :])
                whh_l = wpool.tile([128, 3 * H], BF16, tag="w")
                nc.sync.dma_start(whh_l[:], io["whhT_lo"][ts(c, 128), :])
                first, last = c == 0, c == 7
                # 3-term split: hi@Whi, hi@Wlo, lo@Whi
                ih_terms = [(xT_h, wih_h), (xT_h, wih_l), (xT_l, wih_h)]
                hh_terms = [(hT_h, whh_h), (hT_h, whh_l), (hT_l, whh_h)]
                for half in range(2):
                    nsl = ds(half * 512, 512)
                    for ti, (lt, rt) in enumerate(ih_terms):
                        nc.tensor.matmul(ps_r[:, nsl], lt[:, c, :],
                                         rt[:, ds(half * 512, 512)],
                                         start=first and ti == 0, stop=False)
                        nc.tensor.matmul(ps_z[:, nsl], lt[:, c, :],
                                         rt[:, ds(H + half * 512, 512)],
                                         start=first and ti == 0, stop=False)
                        nc.tensor.matmul(ps_in[:, nsl], lt[:, c, :],
                                         rt[:, ds(2 * H + half * 512, 512)],
                                         start=first and ti == 0,
                                         stop=last and ti == 2)
                    for ti, (lt, rt) in enumerate(hh_terms):
                        nc.tensor.matmul(ps_r[:, nsl], lt[:, c, :],
                                         rt[:, ds(half * 512, 512)],
                                         start=False, stop=last and ti == 2)
                        nc.tensor.matmul(ps_z[:, nsl], lt[:, c, :],
                                         rt[:, ds(H + half * 512, 512)],
                                         start=False, stop=last and ti == 2)
                        nc.tensor.matmul(ps_hn[:, nsl], lt[:, c, :],
                                         rt[:, ds(2 * H + half * 512, 512)],
                                         start=first and ti == 0,
                                         stop=last and ti == 2)

            # gates
            r_sb = sb.tile([BS, H], F32, tag="r_sb")
            z_sb = sb.tile([BS, H], F32, tag="z_sb")
            n_sb = sb.tile([BS, H], F32, tag="n_sb")
            rnn_f = sb.tile([BS, H], F32, tag="rnn_f")
            t1 = sb.tile([BS, H], F32, tag="t1")
            t2 = sb.tile([BS, H], F32, tag="t2")
            t3 = sb.tile([BS, H], F32, tag="t3")
            nc.vector.tensor_tensor(t1[:], ps_r[:], biases["bias_rz"][:, 0:H], OP.add)
            nc.scalar.activation(r_sb[:], t1[:], AF.Sigmoid)
            nc.vector.tensor_tensor(t2[:], ps_z[:], biases["bias_rz"][:, H:2 * H], OP.add)
            nc.scalar.activation(z_sb[:], t2[:], AF.Sigmoid)
            nc.vector.tensor_tensor(t3[:], ps_hn[:], biases["bias_hn"][:], OP.add)
            nc.vector.tensor_tensor(t3[:], r_sb[:], t3[:], OP.mult)
            nc.vector.tensor_tensor(t1[:], ps_in[:], biases["bias_in"][:], OP.add)
            nc.vector.tensor_tensor(t1[:], t1[:], t3[:], OP.add)
            nc.scalar.activation(n_sb[:], t1[:], AF.Tanh)
            nc.vector.tensor_tensor(t2[:], h_nat[:], n_sb[:], OP.subtract)
            nc.vector.tensor_tensor(t2[:], z_sb[:], t2[:], OP.mult)
            nc.vector.tensor_tensor(rnn_f[:], n_sb[:], t2[:], OP.add)

        nc.sync.dma_start(io["hid"][:], rnn_f[:])
        # hi/lo split of rnn (t1 reused as f32 scratch)
        nc.vector.tensor_copy(rnn_h[0:BS, :], rnn_f[:])
        nc.vector.tensor_copy(t1[:], rnn_h[0:BS, :])
        nc.vector.tensor_tensor(t1[:], rnn_f[:], t1[:], OP.subtract)
        nc.vector.tensor_copy(rnn_l[0:BS, :], t1[:])

        # ================= Phase B1: rnn^T, q, q^T =================
        rnnT_h = sb.tile([128, 8, BS], BF16, tag="rnnT_h")
        rnnT_l = sb.tile([128, 8, BS], BF16, tag="rnnT_l")
        qT_h = sb.tile([128, 8, BS], BF16, tag="qT_h")
        qT_l = sb.tile([128, 8, BS], BF16, tag="qT_l")
        with tc.tile_pool(name="psB1", bufs=1, space="PSUM") as psB1:
            def transp(src_pad, dst, c, pool, half_tags=("tpa", "tpb")):
                tp = pool.tile([128, 128], BF16, tag=half_tags[c % 2])
                nc.tensor.transpose(tp[:], src_pad[:, ts(c, 128)], ident[:])
                nc.vector.tensor_copy(dst[:, c, :], tp[:, 0:BS])

            for c in range(8):
                transp(rnn_h, rnnT_h, c, psB1)
                transp(rnn_l, rnnT_l, c, psB1)
            ps_q = psB1.tile([BS, H], F32, tag="q")
            for c in range(8):
                wa_h = wpool.tile([128, H], BF16, tag="w1")
                nc.sync.dma_start(wa_h[:], io["wattn_hi"][ts(c, 128), :])
                wa_l = wpool.tile([128, H], BF16, tag="w1")
                nc.sync.dma_start(wa_l[:], io["wattn_lo"][ts(c, 128), :])
                terms = [(rnnT_h, wa_h), (rnnT_h, wa_l), (rnnT_l, wa_h)]
                for half in range(2):
                    for ti, (lt, rt) in enumerate(terms):
                        nc.tensor.matmul(ps_q[:, ds(half * 512, 512)], lt[:, c, :],
                                         rt[:, ds(half * 512, 512)],
                                         start=(c == 0 and ti == 0),
                                         stop=(c == 7 and ti == 2))
            q_f = sb.tile([BS, H], F32, tag="q_f")
            t1b = sb.tile([BS, H], F32, tag="t2")
            nc.vector.tensor_copy(q_f[:], ps_q[:])
            nc.vector.tensor_copy(q_h[0:BS, :], q_f[:])
            nc.vector.tensor_copy(t1b[:], q_h[0:BS, :])
            nc.vector.tensor_tensor(t1b[:], q_f[:], t1b[:], OP.subtract)
            nc.vector.tensor_copy(q_l[0:BS, :], t1b[:])
            for c in range(8):
                transp(q_h, qT_h, c, psB1)
                transp(q_l, qT_l, c, psB1)

        # ================= Phase B2: energies + softmax + context =================
        e_sb = sb.tile([BS, S], F32)
        wT = sb.tile([128, 4, BS], BF16, tag="wT")
        with tc.tile_pool(name="psB2", bufs=1, space="PSUM") as psB2:
            for b in range(BS):
                et_h = epool.tile([128, 8, 512], BF16, tag="enc")
                nc.sync.dma_start(
                    et_h[:], io["enc_t_hi"][b].rearrange("(c p) s -> p c s", p=128))
                et_l = epool.tile([128, 8, 512], BF16, tag="enc")
                nc.sync.dma_start(
                    et_l[:], io["enc_t_lo"][b].rearrange("(c p) s -> p c s", p=128))
                ps_e = psB2.tile([1, S], F32, tag="ea" if b % 2 == 0 else "eb")
                terms = [(qT_h, et_h), (qT_h, et_l), (qT_l, et_h)]
                for c in range(8):
                    for ti, (lt, rt) in enumerate(terms):
                        nc.tensor.matmul(ps_e[:], lt[:, c, b:b + 1], rt[:, c, :],
                                         start=(c == 0 and ti == 0),
                                         stop=(c == 7 and ti == 2))
                e_row = rows.tile([1, S], F32, tag="e_row")
                nc.vector.tensor_copy(e_row[:], ps_e[:])
                nc.sync.dma_start(e_sb[b:b + 1, :], e_row[:])

            # softmax over s (per-partition row b)
            negmax = sb.tile([BS, 1], F32)
            nc.vector.tensor_reduce(out=negmax[:], in_=e_sb[:],
                                    op=OP.max, axis=mybir.AxisListType.X,
                                    negate=True)
            denom = sb.tile([BS, 1], F32)
            w_f = sb.tile([BS, S], F32, tag="w_f")
            nc.scalar.activation(w_f[:], e_sb[:], AF.Exp,
                                 bias=negmax[:], accum_out=denom[:])
            rec = sb.tile([BS, 1], F32)
            nc.vector.reciprocal(rec[:], denom[:])
            nc.vector.tensor_scalar_mul(w_f[:], w_f[:], rec[:])
            nc.sync.dma_start(io["attn"][:], w_f[:])
            nc.vector.tensor_copy(w_16[0:BS, :], w_f[:])

            for c in range(4):
                tp = psB2.tile([128, 128], BF16, tag="tpa" if c % 2 == 0 else "tpb")
                nc.tensor.transpose(tp[:], w_16[:, ts(c, 128)], ident[:])
                nc.vector.tensor_copy(wT[:, c, :], tp[:, 0:BS])

            for b in range(BS):
                en = epool.tile([128, 4, 1024], BF16, tag="enc")
                nc.sync.dma_start(
                    en[:], io["enc_n16"][b].rearrange("(c p) d -> p c d", p=128))
                ps_c = psB2.tile([1, H], F32, tag="ca" if b % 2 == 0 else "cb")
                for c in range(4):
                    for half in range(2):
                        nc.tensor.matmul(ps_c[:, ds(half * 512, 512)],
                                         wT[:, c, b:b + 1],
                                         en[:, c, ds(half * 512, 512)],
                                         start=(c == 0), stop=(c == 3))
                c_row = rows.tile([1, H], BF16, tag="c_row")
                nc.scalar.copy(c_row[:], ps_c[:])
                nc.sync.dma_start(ctx_16[b:b + 1, :], c_row[:])

        # ================= Phase C: output head =================
        with tc.tile_pool(name="psC", bufs=1, space="PSUM") as psC:
            ctxT = sb.tile([128, 8, BS], BF16, tag="ctxT")
            for c in range(8):
                tp = psC.tile([128, 128], BF16, tag="tpa" if c % 2 == 0 else "tpb")
                nc.tensor.transpose(tp[:], ctx_16[:, ts(c, 128)], ident[:])
                nc.vector.tensor_copy(ctxT[:, c, :], tp[:, 0:BS])

            ps_cat = psC.tile([BS, H], F32, tag="cat")
            for g in range(16):
                wc_g = wpool.tile([128, H], BF16, tag="w1")
                nc.sync.dma_start(wc_g[:], io["wcatT16"][ts(g, 128), :])
                lhsT = rnnT_h[:, g, :] if g < 8 else ctxT[:, g - 8, :]
                for half in range(2):
                    nc.tensor.matmul(ps_cat[:, ds(half * 512, 512)], lhsT,
                                     wc_g[:, ds(half * 512, 512)],
                                     start=(g == 0), stop=(g == 15))
            t4 = sb.tile([BS, H], F32, tag="t1")
            nc.vector.tensor_tensor(t4[:], ps_cat[:], biases["bias_cat"][:], OP.add)
            nc.scalar.activation(cat_16[0:BS, :], t4[:], AF.Tanh)

            catT = sb.tile([128, 8, BS], BF16, tag="catT")
            for c in range(8):
                tp = psC.tile([128, 128], BF16, tag="tpa" if c % 2 == 0 else "tpb")
                nc.tensor.transpose(tp[:], cat_16[:, ts(c, 128)], ident[:])
                nc.vector.tensor_copy(catT[:, c, :], tp[:, 0:BS])

            ps_out = psC.tile([BS, H], F32, tag="out")
            for c in range(8):
                wo_c = wpool.tile([128, H], BF16, tag="w1")
                nc.sync.dma_start(wo_c[:], io["woutT16"][ts(c, 128), :])
                for half in range(2):
                    nc.tensor.matmul(ps_out[:, ds(half * 512, 512)], catT[:, c, :],
                                     wo_c[:, ds(half * 512, 512)],
                                     start=(c == 0), stop=(c == 7))
            t5 = sb.tile([BS, H], F32, tag="t2")
            nc.vector.tensor_tensor(t5[:], ps_out[:], biases["bias_out"][:], OP.add)
            out_sb = sb.tile([BS, H], F32, tag="t3")
            nc.scalar.activation(out_sb[:], t5[:], AF.Sigmoid)
            nc.sync.dma_start(io["out"][:], out_sb[:])


def build_nc(reps=1):
    if ("nc", reps) in _cached:
        return _cached[("nc", reps)]
    nc = bacc.Bacc("TRN2", target_bir_lowering=False, debug=False,
                   num_devices=NCORES)
    io = {}
    in_specs = [
        ("xT_hi", [H, BS], BF16), ("xT_lo", [H, BS], BF16),
        ("hT_hi", [H, BS], BF16), ("hT_lo", [H, BS], BF16),
        ("h_nat", [BS, H], F32),
        ("enc_t_hi", [BS, H, S], BF16), ("enc_t_lo", [BS, H, S], BF16),
        ("enc_n16", [BS, S, H], BF16),
        ("wihT_hi", [H, 3 * H], BF16), ("wihT_lo", [H, 3 * H], BF16),
        ("whhT_hi", [H, 3 * H], BF16), ("whhT_lo", [H, 3 * H], BF16),
        ("wattn_hi", [H, H], BF16), ("wattn_lo", [H, H], BF16),
        ("wcatT16", [2 * H, H], BF16), ("woutT16", [H, H], BF16),
        ("bias_rz", [BS, 2 * H], F32), ("bias_in", [BS, H], F32),
        ("bias_hn", [BS, H], F32),
        ("bias_cat", [BS, H], F32), ("bias_out", [BS, H], F32),
    ]
    for name, shape, dt in in_specs:
        io[name] = nc.dram_tensor(name, shape, dt, kind="ExternalInput").ap()
    for name, shape in [("out", [BS, H]), ("hid", [BS, H]), ("attn", [BS, S])]:
        io[name] = nc.dram_tensor(name, shape, F32, kind="ExternalOutput").ap()
    with tile.TileContext(nc) as tc:
        for _ in range(reps):
            _build_kernel(tc, io)
    nc.compile()
    _cached[("nc", reps)] = nc
    return nc


def _split(x):
    import ml_dtypes
    hi = np.ascontiguousarray(x, dtype=np.float32).astype(ml_dtypes.bfloat16)
    lo = (np.ascontiguousarray(x, dtype=np.float32)
          - hi.astype(np.float32)).astype(ml_dtypes.bfloat16)
    return hi, lo


def make_in_maps(input_seq, last_hidden, encoder_outputs,
                 W_ih, b_ih, W_hh, b_hh, W_attn, b_attn,
                 W_concat, b_concat, W_out, b_out):
    import ml_dtypes
    f = np.float32
    wih_h, wih_l = _split(np.asarray(W_ih, f).T)
    whh_h, whh_l = _split(np.asarray(W_hh, f).T)
    wa_h, wa_l = _split(np.asarray(W_attn, f))
    wcat16 = np.ascontiguousarray(np.asarray(W_concat, f).T).astype(ml_dtypes.bfloat16)
    wout16 = np.ascontiguousarray(np.asarray(W_out, f).T).astype(ml_dtypes.bfloat16)
    b3 = (np.asarray(b_ih, f) + np.asarray(b_hh, f))
    bias_rz = np.tile(b3[None, :2 * H], (BS, 1))
    bias_in = np.tile(np.asarray(b_ih, f)[None, 2 * H:], (BS, 1))
    bias_hn = np.tile(np.asarray(b_hh, f)[None, 2 * H:], (BS, 1))
    bias_cat = np.tile(np.asarray(b_concat, f)[None, :], (BS, 1))
    bias_out = np.tile(np.asarray(b_out, f)[None, :], (BS, 1))
    x = np.asarray(input_seq, f)          # [B, H]
    h = np.asarray(last_hidden, f)[0]     # [B, H]
    enc = np.asarray(encoder_outputs, f)  # [S, B, H]
    enc_bsh = enc.transpose(1, 0, 2)      # [B, S, H]
    enc_bhs = enc.transpose(1, 2, 0)      # [B, H, S]
    in_maps = []
    for core in range(NCORES):
        bsl = slice(core * BS, (core + 1) * BS)
        xT_hi, xT_lo = _split(x[bsl].T)
        hT_hi, hT_lo = _split(h[bsl].T)
        et_hi, et_lo = _split(enc_bhs[bsl])
        en16 = np.ascontiguousarray(enc_bsh[bsl]).astype(ml_dtypes.bfloat16)
        in_maps.append({
            "xT_hi": xT_hi, "xT_lo": xT_lo, "hT_hi": hT_hi, "hT_lo": hT_lo,
            "h_nat": np.ascontiguousarray(h[bsl]),
            "enc_t_hi": et_hi, "enc_t_lo": et_lo, "enc_n16": en16,
            "wihT_hi": wih_h, "wihT_lo": wih_l,
            "whhT_hi": whh_h, "whhT_lo": whh_l,
            "wattn_hi": wa_h, "wattn_lo": wa_l,
            "wcatT16": wcat16, "woutT16": wout16,
            "bias_rz": bias_rz, "bias_in": bias_in, "bias_hn": bias_hn,
            "bias_cat": bias_cat, "bias_out": bias_out,
        })
    return in_maps


def assemble_outputs(results):
    output = np.concatenate([r["out"] for r in results], axis=0)
    hidden = np.concatenate([r["hid"] for r in results], axis=0)[None]
    attn = np.concatenate([r["attn"] for r in results], axis=0)[:, None, :]
    return output, hidden, attn


def kernel(**inputs):
    nc = build_nc()
    in_maps = make_in_maps(**inputs)
    res = run_bass_kernel_spmd(nc, in_maps, core_ids=list(range(NCORES)))
    return assemble_outputs(res.results)


# revision 28
# speedup vs baseline: 569.5584x; 569.5584x over previous
"""AttnDecoderRNN (single GRU step + general attention + output head) on 8 trn2 cores.

Sharding: batch B=128 -> 16 per core for attention/context/output head
(data-parallel), but the GRU + q projection are sharded over the HIDDEN dim
instead (128 cols per core) so the big replicated weights (W_ih, W_hh, W_attn:
28MB/core) shrink to per-core strips (4MB/core total):

  core j computes rnn[:, jslice] for ALL 128 batches (W strips only),
  then locally forms partials   q_part_j   = rnn[:, jslice] @ W_attn[jslice, :]
                                cat_part_j = rnn[:, jslice] @ W_cat.T[jslice, :1024-out]
  and ONE ReduceScatter(add) of [128, 2048] returns exactly this core's
  batch-shard rows of q and of the rnn-half of the concat matmul.
  hidden (= rnn) is emitted as per-core d-slices and reassembled on host.

Precision: fp32 PE matmuls cost 4 cycles/row on trn2; bf16 costs 1.
Everything upstream of the softmax needs ~1e-4 absolute accuracy on the
energies (softmax exponentiates absolute errors), so those matmuls use an
exact bf16 hi+lo split (x = hi + lo, both bf16) and compute
  x@W ~= hi@W_hi + hi@W_lo + lo@W_hi   (3 bf16 passes, fp32 PSUM accumulate)
dropping only the lo@lo term (~1e-5 rel). The post-softmax path (context,
concat/output head) tolerates relative error and runs plain bf16 1-pass.

Math (per core, b = local batch 0..15):
  gi = x @ W_ih.T ; gh = h @ W_hh.T
  r = sig(gi_r+gh_r), z = sig(gi_z+gh_z), n = tanh(i_n + r*h_n)   (biases zero)
  rnn = n + z*(h-n)
  q = rnn @ W_attn          (b_attn shifts energies per-b only -> softmax-invariant)
  e[b,s] = sum_d enc[s,b,d] * q[b,d]
  w = softmax(e) ; ctx[b,:] = sum_s w[b,s] enc[s,b,:]
  out = sig(tanh([rnn,ctx] @ W_cat.T) @ W_out.T)
"""

import numpy as np
from contextlib import ExitStack

import concourse.bass as bass
import concourse.tile as tile
from concourse import bacc, mybir
from concourse.bass import ts, ds
from concourse.bass_utils import run_bass_kernel_spmd
from concourse.masks import make_identity

B, S, H = 128, 512, 1024
NCORES = 8
BS = B // NCORES   # 16  (batch shard)
DS = H // NCORES   # 128 (hidden-dim shard)
F32 = mybir.dt.float32
BF16 = mybir.dt.bfloat16
AF = mybir.ActivationFunctionType
OP = mybir.AluOpType

_cached = {}


def _build_kernel(tc: tile.TileContext, io: dict):
    nc = tc.nc
    with ExitStack() as ctx:
        const = ctx.enter_context(tc.tile_pool(name="const", bufs=1))
        sb = ctx.enter_context(tc.tile_pool(name="sb", bufs=1))
        wpool = ctx.enter_context(tc.tile_pool(name="w", bufs=6))
        epool = ctx.enter_context(tc.tile_pool(name="enc", bufs=5))
        rows = ctx.enter_context(tc.tile_pool(name="rows", bufs=3))

        ident = const.tile([128, 128], BF16)
        make_identity(nc, ident[:])

        def load3(name, shape3, srcname):
            t = sb.tile(shape3, BF16, tag=name, name=name)
            nc.sync.dma_start(
                t[:], io[srcname].rearrange("(c p) n -> p c n", p=128))
            return t

        # full-B inputs (hi/lo bf16), [1024, 128] -> [128, 8, 128]
        xT_h = load3("xT_h", [128, 8, B], "xT_hi")
        xT_l = load3("xT_l", [128, 8, B], "xT_lo")
        hT_h = load3("hT_h", [128, 8, B], "hT_hi")
        hT_l = load3("hT_l", [128, 8, B], "hT_lo")
        # per-core GRU weight strips [1024, 384] -> [128, 8, 384]
        wihs_h = load3("wihs_h", [128, 8, 384], "wih_strip_hi")
        wihs_l = load3("wihs_l", [128, 8, 384], "wih_strip_lo")
        whhs_h = load3("whhs_h", [128, 8, 384], "whh_strip_hi")
        whhs_l = load3("whhs_l", [128, 8, 384], "whh_strip_lo")
        h_slice = sb.tile([B, DS], F32)
        nc.sync.dma_start(h_slice[:], io["h_slice"][:])
        was_h = sb.tile([128, H], BF16, tag="was_h")
        nc.sync.dma_start(was_h[:], io["wattn_strip_hi"][:])
        was_l = sb.tile([128, H], BF16, tag="was_l")
        nc.sync.dma_start(was_l[:], io["wattn_strip_lo"][:])
        wcrs = sb.tile([128, H], BF16, tag="wcrs")
        nc.sync.dma_start(wcrs[:], io["wcat_rnn_strip"][:])

        # padded staging tiles for PE transposes of per-bshard vectors
        q_h = sb.tile([128, H], BF16, tag="q_h")
        q_l = sb.tile([128, H], BF16, tag="q_l")
        ctx_16 = sb.tile([128, H], BF16, tag="ctx_16")
        cat_16 = sb.tile([128, H], BF16, tag="cat_16")
        for t in (q_h, q_l, ctx_16, cat_16):
            nc.vector.memset(t[:], 0.0)

        # ========== Phase A: hidden-sharded GRU (all B, DS cols) ==========
        with tc.tile_pool(name="psA", bufs=1, space="PSUM") as psA:
            ps_rz = psA.tile([B, 2 * DS], F32, tag="rz")
            ps_in = psA.tile([B, DS], F32, tag="in")
            ps_hn = psA.tile([B, DS], F32, tag="hn")
            ih_terms = [(xT_h, wihs_h), (xT_h, wihs_l), (xT_l, wihs_h)]
            hh_terms = [(hT_h, whhs_h), (hT_h, whhs_l), (hT_l, whhs_h)]
            for c in range(8):
                first, last = c == 0, c == 7
                for ti, (lt, rt) in enumerate(ih_terms):
                    st, sp = first and ti == 0, last and ti == 2
                    nc.tensor.matmul(ps_rz[:], lt[:, c, :], rt[:, c, 0:2 * DS],
                                     start=st, stop=False)
                    nc.tensor.matmul(ps_in[:], lt[:, c, :], rt[:, c, 2 * DS:],
                                     start=st, stop=sp)
                for ti, (lt, rt) in enumerate(hh_terms):
                    st, sp = first and ti == 0, last and ti == 2
                    nc.tensor.matmul(ps_rz[:], lt[:, c, :], rt[:, c, 0:2 * DS],
                                     start=False, stop=sp)
                    nc.tensor.matmul(ps_hn[:], lt[:, c, :], rt[:, c, 2 * DS:],
                                     start=st, stop=sp)

            # gates on [128B, 128] tiles (biases are all-zero per spec)
            r_s = sb.tile([B, DS], F32, tag="r_s")
            z_s = sb.tile([B, DS], F32, tag="z_s")
            n_s = sb.tile([B, DS], F32, tag="n_s")
            rnn_j = sb.tile([B, DS], F32, tag="rnn_j")
            g1 = sb.tile([B, DS], F32, tag="g1")
            nc.scalar.activation(r_s[:], ps_rz[:, 0:DS], AF.Sigmoid)
            nc.scalar.activation(z_s[:], ps_rz[:, DS:2 * DS], AF.Sigmoid)
            nc.vector.tensor_tensor(g1[:], r_s[:], ps_hn[:], OP.mult)
            nc.vector.tensor_tensor(g1[:], g1[:], ps_in[:], OP.add)
            nc.scalar.activation(n_s[:], g1[:], AF.Tanh)
            nc.vector.tensor_tensor(g1[:], h_slice[:], n_s[:], OP.subtract)
            nc.vector.tensor_tensor(g1[:], z_s[:], g1[:], OP.mult)
            nc.vector.tensor_tensor(rnn_j[:], n_s[:], g1[:], OP.add)

        nc.sync.dma_start(io["hidpart"][:], rnn_j[:])

        # ========== Phase B: local q/cat partials + ReduceScatter ==========
        with tc.tile_pool(name="psB", bufs=1, space="PSUM") as psB:
            rj_h = sb.tile([B, DS], BF16, tag="rj_h")
            rj_l = sb.tile([B, DS], BF16, tag="rj_l")
            g2 = sb.tile([B, DS], F32, tag="g2")
            nc.vector.tensor_copy(rj_h[:], rnn_j[:])
            nc.vector.tensor_copy(g2[:], rj_h[:])
            nc.vector.tensor_tensor(g2[:], rnn_j[:], g2[:], OP.subtract)
            nc.vector.tensor_copy(rj_l[:], g2[:])
            rjT_h = sb.tile([128, B], BF16, tag="rjT_h")
            rjT_l = sb.tile([128, B], BF16, tag="rjT_l")
            for src, dst in [(rj_h, rjT_h), (rj_l, rjT_l)]:
                tp = psB.tile([128, 128], BF16, tag="tpa", name="tp")
                nc.tensor.transpose(tp[:], src[:], ident[:])
                nc.vector.tensor_copy(dst[:], tp[:])

            ps_qp = psB.tile([B, H], F32, tag="qp")
            ps_cr = psB.tile([B, H], F32, tag="cr")
            for half in range(2):
                nsl = ds(half * 512, 512)
                for ti, (lt, rt) in enumerate(
                        [(rjT_h, was_h), (rjT_h, was_l), (rjT_l, was_h)]):
                    nc.tensor.matmul(ps_qp[:, nsl], lt[:], rt[:, nsl],
                                     start=ti == 0, stop=ti == 2)
                nc.tensor.matmul(ps_cr[:, nsl], rjT_h[:], wcrs[:, nsl],
                                 start=True, stop=True)
            qp_f = sb.tile([B, H], F32, tag="qp_f")
            crp_f = sb.tile([B, H], F32, tag="crp_f")
            nc.vector.tensor_copy(qp_f[:], ps_qp[:])
            nc.scalar.copy(crp_f[:], ps_cr[:])
            nc.sync.dma_start(io["cc_in"][:, 0:H], qp_f[:])
            nc.sync.dma_start(io["cc_in"][:, H:2 * H], crp_f[:])
            nc.gpsimd.collective_compute(
                "ReduceScatter", OP.add,
                replica_groups=[list(range(NCORES))],
                ins=[io["cc_in"][:]], outs=[io["cc_out"][:]],
            )
            q_f = sb.tile([BS, H], F32, tag="q_f")
            catrnn_f = sb.tile([BS, H], F32, tag="catrnn_f")
            nc.sync.dma_start(q_f[:], io["cc_out"][:, 0:H])
            nc.sync.dma_start(catrnn_f[:], io["cc_out"][:, H:2 * H])

            # split q (per-bshard) hi/lo and transpose
            g3 = sb.tile([BS, H], F32, tag="g3")
            nc.vector.tensor_copy(q_h[0:BS, :], q_f[:])
            nc.vector.tensor_copy(g3[:], q_h[0:BS, :])
            nc.vector.tensor_tensor(g3[:], q_f[:], g3[:], OP.subtract)
            nc.vector.tensor_copy(q_l[0:BS, :], g3[:])
            qT_h = sb.tile([128, 8, BS], BF16, tag="qT_h")
            qT_l = sb.tile([128, 8, BS], BF16, tag="qT_l")
            for c in range(8):
                for src, dst in [(q_h, qT_h), (q_l, qT_l)]:
                    tp = psB.tile([128, 128], BF16,
                                  tag="tpa" if c % 2 == 0 else "tpb", name="tp")
                    nc.tensor.transpose(tp[:], src[:, ts(c, 128)], ident[:])
                    nc.vector.tensor_copy(dst[:, c, :], tp[:, 0:BS])

        # ========== Phase C: energies + softmax + context (grouped) ==========
        GRP, NG = 4, 4
        with tc.tile_pool(name="psC", bufs=1, space="PSUM") as psC:
            for g in range(NG):
                e_g = sb.tile([GRP, S], F32, tag=f"e_g{g}", name=f"e_g{g}")
                w16_g = sb.tile([128, S], BF16, tag=f"w16_{g}", name=f"w16_{g}")
                nc.vector.memset(w16_g[:], 0.0)
                wT_g = sb.tile([128, 4, GRP], BF16, tag=f"wT_{g}", name=f"wT_{g}")
                for j in range(GRP):
                    b = g * GRP + j
                    et_h = epool.tile([128, 8, 512], BF16, tag="enc", name="et_h")
                    nc.sync.dma_start(
                        et_h[:], io["enc_t_hi"][b].rearrange("(c p) s -> p c s", p=128))
                    et_l = epool.tile([128, 8, 512], BF16, tag="enc", name="et_l")
                    nc.sync.dma_start(
                        et_l[:], io["enc_t_lo"][b].rearrange("(c p) s -> p c s", p=128))
                    ps_e = psC.tile([1, S], F32, tag="ea" if b % 2 == 0 else "eb",
                                    name="ps_e")
                    terms = [(qT_h, et_h), (qT_h, et_l), (qT_l, et_h)]
                    for c in range(8):
                        for ti, (lt, rt) in enumerate(terms):
                            nc.tensor.matmul(ps_e[:], lt[:, c, b:b + 1], rt[:, c, :],
                                             start=(c == 0 and ti == 0),
                                             stop=(c == 7 and ti == 2))
                    e_row = rows.tile([1, S], F32, tag="e_row", name="e_row")
                    nc.vector.tensor_copy(e_row[:], ps_e[:])
                    nc.sync.dma_start(e_g[j:j + 1, :], e_row[:])

                negmax = sb.tile([GRP, 1], F32, tag=f"nm{g}", name=f"nm{g}")
                nc.vector.tensor_reduce(out=negmax[:], in_=e_g[:],
                                        op=OP.max, axis=mybir.AxisListType.X,
                                        negate=True)
                denom = sb.tile([GRP, 1], F32, tag=f"dn{g}", name=f"dn{g}")
                w_fg = sb.tile([GRP, S], F32, tag=f"wf{g}", name=f"wf{g}")
                nc.scalar.activation(w_fg[:], e_g[:], AF.Exp,
                                     bias=negmax[:], accum_out=denom[:])
                rec = sb.tile([GRP, 1], F32, tag=f"rc{g}", name=f"rc{g}")
                nc.vector.reciprocal(rec[:], denom[:])
                nc.vector.tensor_scalar_mul(w_fg[:], w_fg[:], rec[:])
                nc.sync.dma_start(io["attn"][g * GRP:(g + 1) * GRP, :], w_fg[:])
                nc.vector.tensor_copy(w16_g[0:GRP, :], w_fg[:])
                for c in range(4):
                    tp = psC.tile([128, 128], BF16,
                                  tag="tpa" if c % 2 == 0 else "tpb", name="tp")
                    nc.tensor.transpose(tp[:], w16_g[:, ts(c, 128)], ident[:])
                    nc.vector.tensor_copy(wT_g[:, c, :], tp[:, 0:GRP])

                for j in range(GRP):
                    b = g * GRP + j
                    en = epool.tile([128, 4, 1024], BF16, tag="encN", name="en")
                    nc.sync.dma_start(
                        en[:], io["enc_n16"][b].rearrange("(c p) d -> p c d", p=128))
                    ps_c = psC.tile([1, H], F32, tag="ca" if b % 2 == 0 else "cb",
                                    name="ps_c")
                    for c in range(4):
                        for half in range(2):
                            nc.tensor.matmul(ps_c[:, ds(half * 512, 512)],
                                             wT_g[:, c, j:j + 1],
                                             en[:, c, ds(half * 512, 512)],
                                             start=(c == 0), stop=(c == 3))
                    c_row = rows.tile([1, H], BF16, tag="c_row", name="c_row")
                    nc.scalar.copy(c_row[:], ps_c[:])
                    nc.sync.dma_start(ctx_16[b:b + 1, :], c_row[:])

        # ========== Phase D: output head ==========
        with tc.tile_pool(name="psD", bufs=1, space="PSUM") as psD:
            ctxT = sb.tile([128, 8, BS], BF16, tag="ctxT")
            for c in range(8):
                tp = psD.tile([128, 128], BF16, tag="tpa" if c % 2 == 0 else "tpb",
                              name="tp")
                nc.tensor.transpose(tp[:], ctx_16[:, ts(c, 128)], ident[:])
                nc.vector.tensor_copy(ctxT[:, c, :], tp[:, 0:BS])

            ps_cat = psD.tile([BS, H], F32, tag="cat")
            for g in range(8):  # ctx half of the concat matmul
                wc_g = wpool.tile([128, H], BF16, tag="w1", name="wc_g")
                nc.sync.dma_start(wc_g[:], io["wcat_ctx16"][ts(g, 128), :])
                for half in range(2):
                    nc.tensor.matmul(ps_cat[:, ds(half * 512, 512)], ctxT[:, g, :],
                                     wc_g[:, ds(half * 512, 512)],
                                     start=(g == 0), stop=(g == 7))
            t4 = sb.tile([BS, H], F32, tag="t4")
            nc.vector.tensor_tensor(t4[:], ps_cat[:], catrnn_f[:], OP.add)
            nc.scalar.activation(cat_16[0:BS, :], t4[:], AF.Tanh)

            catT = sb.tile([128, 8, BS], BF16, tag="catT")
            for c in range(8):
                tp = psD.tile([128, 128], BF16, tag="tpa" if c % 2 == 0 else "tpb",
                              name="tp")
                nc.tensor.transpose(tp[:], cat_16[:, ts(c, 128)], ident[:])
                nc.vector.tensor_copy(catT[:, c, :], tp[:, 0:BS])

            ps_out = psD.tile([BS, H], F32, tag="out")
            for c in range(8):
                wo_c = wpool.tile([128, H], BF16, tag="w1", name="wo_c")
                nc.sync.dma_start(wo_c[:], io["woutT16"][ts(c, 128), :])
                for half in range(2):
                    nc.tensor.matmul(ps_out[:, ds(half * 512, 512)], catT[:, c, :],
                                     wo_c[:, ds(half * 512, 512)],
                                     start=(c == 0), stop=(c == 7))
            out_sb = sb.tile([BS, H], F32, tag="out_sb")
            nc.scalar.activation(out_sb[:], ps_out[:], AF.Sigmoid)
            nc.sync.dma_start(io["out"][:], out_sb[:])


def build_nc(reps=1):
    if ("nc", reps) in _cached:
        return _cached[("nc", reps)]
    nc = bacc.Bacc("TRN2", target_bir_lowering=False, debug=False,
                   num_devices=NCORES)
    io = {}
    in_specs = [
        ("xT_hi", [H, B], BF16), ("xT_lo", [H, B], BF16),
        ("hT_hi", [H, B], BF16), ("hT_lo", [H, B], BF16),
        ("h_slice", [B, DS], F32),
        ("wih_strip_hi", [H, 3 * DS], BF16), ("wih_strip_lo", [H, 3 * DS], BF16),
        ("whh_strip_hi", [H, 3 * DS], BF16), ("whh_strip_lo", [H, 3 * DS], BF16),
        ("wattn_strip_hi", [DS, H], BF16), ("wattn_strip_lo", [DS, H], BF16),
        ("wcat_rnn_strip", [DS, H], BF16),
        ("wcat_ctx16", [H, H], BF16), ("woutT16", [H, H], BF16),
        ("enc_t_hi", [BS, H, S], BF16), ("enc_t_lo", [BS, H, S], BF16),
        ("enc_n16", [BS, S, H], BF16),
    ]
    for name, shape, dt in in_specs:
        io[name] = nc.dram_tensor(name, shape, dt, kind="ExternalInput").ap()
    for name, shape in [("out", [BS, H]), ("hidpart", [B, DS]),
                        ("attn", [BS, S])]:
        io[name] = nc.dram_tensor(name, shape, F32, kind="ExternalOutput").ap()
    io["cc_in"] = nc.dram_tensor("cc_in", [B, 2 * H], F32).ap()
    io["cc_out"] = nc.dram_tensor("cc_out", [BS, 2 * H], F32).ap()
    with tile.TileContext(nc) as tc:
        for _ in range(reps):
            _build_kernel(tc, io)
    nc.compile()
    _cached[("nc", reps)] = nc
    return nc


def _split(x):
    import ml_dtypes
    x = np.ascontiguousarray(x, dtype=np.float32)
    hi = x.astype(ml_dtypes.bfloat16)
    lo = (x - hi.astype(np.float32)).astype(ml_dtypes.bfloat16)
    return hi, lo


def make_in_maps(input_seq, last_hidden, encoder_outputs,
                 W_ih, b_ih, W_hh, b_hh, W_attn, b_attn,
                 W_concat, b_concat, W_out, b_out):
    import ml_dtypes
    f = np.float32
    for bz in (b_ih, b_hh, b_attn, b_concat, b_out):
        assert not np.any(np.asarray(bz)), "kernel assumes zero biases (per spec)"
    wihT = np.asarray(W_ih, f).T   # [H, 3H]
    whhT = np.asarray(W_hh, f).T
    wattn = np.asarray(W_attn, f)  # [H, H]
    wcatT = np.asarray(W_concat, f).T  # [2H, H]
    wout16 = np.ascontiguousarray(np.asarray(W_out, f).T).astype(ml_dtypes.bfloat16)
    wcat_ctx16 = np.ascontiguousarray(wcatT[H:]).astype(ml_dtypes.bfloat16)
    x = np.asarray(input_seq, f)
    h = np.asarray(last_hidden, f)[0]
    enc = np.asarray(encoder_outputs, f)
    enc_bsh = enc.transpose(1, 0, 2)
    enc_bhs = enc.transpose(1, 2, 0)
    xT_hi, xT_lo = _split(x.T)
    hT_hi, hT_lo = _split(h.T)
    in_maps = []
    for core in range(NCORES):
        bsl = slice(core * BS, (core + 1) * BS)
        dsl = slice(core * DS, (core + 1) * DS)
        wih_strip = np.concatenate(
            [wihT[:, dsl], wihT[:, H + core * DS:H + (core + 1) * DS],
             wihT[:, 2 * H + core * DS:2 * H + (core + 1) * DS]], axis=1)
        whh_strip = np.concatenate(
            [whhT[:, dsl], whhT[:, H + core * DS:H + (core + 1) * DS],
             whhT[:, 2 * H + core * DS:2 * H + (core + 1) * DS]], axis=1)
        ws_hi, ws_lo = _split(wih_strip)
        wh_hi, wh_lo = _split(whh_strip)
        wa_hi, wa_lo = _split(wattn[dsl, :])
        et_hi, et_lo = _split(enc_bhs[bsl])
        in_maps.append({
            "xT_hi": xT_hi, "xT_lo": xT_lo, "hT_hi": hT_hi, "hT_lo": hT_lo,
            "h_slice": np.ascontiguousarray(h[:, dsl]),
            "wih_strip_hi": ws_hi, "wih_strip_lo": ws_lo,
            "whh_strip_hi": wh_hi, "whh_strip_lo": wh_lo,
            "wattn_strip_hi": wa_hi, "wattn_strip_lo": wa_lo,
            "wcat_rnn_strip": np.ascontiguousarray(
                wcatT[dsl]).astype(ml_dtypes.bfloat16),
            "wcat_ctx16": wcat_ctx16, "woutT16": wout16,
            "enc_t_hi": et_hi, "enc_t_lo": et_lo,
            "enc_n16": np.ascontiguousarray(enc_bsh[bsl]).astype(ml_dtypes.bfloat16),
        })
    return in_maps


def assemble_outputs(results):
    output = np.concatenate([r["out"] for r in results], axis=0)
    hidden = np.concatenate([r["hidpart"] for r in results], axis=1)[None]
    attn = np.concatenate([r["attn"] for r in results], axis=0)[:, None, :]
    return output, hidden, attn


def kernel(**inputs):
    nc = build_nc()
    in_maps = make_in_maps(**inputs)
    res = run_bass_kernel_spmd(nc, in_maps, core_ids=list(range(NCORES)))
    return assemble_outputs(res.results)
